# revision 5
# baseline (speedup 1.0000x reference)
"""BandSplit Trainium2 kernel, v4 (XBAR DMA-transpose load).

Math (per sample b, band j covering flat-channel segment [q0, q0+w)):
  x viewed as (T, 962); GroupNorm over (T, w) per band, then per-band
  1x1 conv:  out_j = fw_j @ xn_j^T + fb_j.

Folding: out[c,t] = rstd_j*(fwn_j @ x)[c,t] + (c1[c,j] - mu_j*rstd_j*c2[c,j])
with fwn = fw*nw (HOST, bf16), c1 = fb + fw@nb, c2 = fw@nw (HOST, f32);
only the scalars mu_j, rstd_j are computed on device and applied in the
PSUM->staging copy as a fused scale+bias.

v4 load path: the host ships x chunk-padded as (T, 10*128) bf16 and each
of the 10 band-aligned chunks is brought into SBUF directly transposed by
the XBAR DMA (InstDmaTransposeAnt, 16x128 tiles; T=2000 = 125*16). This
removes the PE transposes and all PSUM->SBUF copies from the load phase;
bn_stats for a chunk starts as soon as its single DMA lands, so the
stats gate moves ~12us earlier.

Output: per (t-chunk, band) bf16 matmul K=w into PSUM; ACT/DVE fused
copy out = psum*rstd_j + btot[:,j] into band-interleaved staging; one
DMA per t-chunk writes (C, tc, 34) contiguously, graduated chunk sizes
so the write stream starts early and stays dense.
"""
import numpy as np

GROUPS = [(0, 1, 5), (5, 19, 4), (81, 6, 10), (141, 7, 40), (421, 1, 60)]
B, C, T, Q, NB = 8, 128, 2000, 962, 34
EPS = 1e-5
TCS = [100, 200, 300, 400, 400, 400, 200]   # output t-chunks (graduated)
assert sum(TCS) == T
# bn_stats windows (start, len); <=512 (HW limit), count-weighted by aggr
WINS = [(0, 500), (500, 500), (1000, 500), (1500, 500)]

BANDS = []
for _g, (_off, _n, _s) in enumerate(GROUPS):
    for _i in range(_n):
        BANDS.append((2 * _off + _i * 2 * _s, 2 * _s, _g, _i))
assert len(BANDS) == NB and BANDS[-1][0] + BANDS[-1][1] == Q

# band-aligned chunks: consecutive bands packed into <=128 rows
CHUNKS = []   # (q0, rows, [band indices])
_cur, _rows, _q0 = [], 0, 0
for _j, (_qb, _w, _g, _i) in enumerate(BANDS):
    if _rows + _w > 128:
        CHUNKS.append((_q0, _rows, _cur))
        _cur, _rows, _q0 = [], 0, _qb
    _cur.append(_j)
    _rows += _w
CHUNKS.append((_q0, _rows, _cur))
NCH = len(CHUNKS)
CHUNK_OF = {}
for _c, (_q0, _r, _bl) in enumerate(CHUNKS):
    for _j in _bl:
        CHUNK_OF[_j] = _c
XQ = NCH * 128   # chunk-padded x width

# packed const layout (f32 columns); fwn first, late-needed, so the
# small early DMA (ind..eps) can precede it
_off = 0
CONST_OFF = {}
def _creserve(name, cols):
    global _off
    CONST_OFF[name] = _off
    _off += cols
for _j in range(NB):
    _creserve(f"fwn{_j}", 64)             # (128,128) bf16 each
for _c in range(NCH):
    _creserve(f"ind{_c}", NB)             # (128,34) f32
_creserve("c1t", NB)
_creserve("c2t", NB)
_creserve("ones34", C)                    # rows 0..33 used
_creserve("eye34", NB)
_creserve("invc", 1)
_creserve("epsv", 1)
NCONST = _off
CST_SPLIT = CONST_OFF["ind0"]             # [SPLIT:] loads first


def _align(r0, r1):
    """Largest legal quadrant base <= r0 covering [r0, r1)."""
    for base in (64, 32, 0):
        if base > r0:
            continue
        K = r1 - base
        if K <= 32:
            return base, K
        if K <= 64 and base in (0, 64):
            return base, K
        if base == 0:
            return 0, K
    raise AssertionError((r0, r1))


def _band_window(j):
    q0, w, _g, _i = BANDS[j]
    c = CHUNK_OF[j]
    r0 = q0 - CHUNKS[c][0]
    base, K = _align(r0, r0 + w)
    return c, base, K


def host_constants(inputs):
    import concourse.mybir as mybir
    bf16 = mybir.dt.np(mybir.dt.bfloat16)

    fws = [np.asarray(inputs[f"fw{g}"], np.float32) for g in range(5)]
    nws = [np.asarray(inputs[f"nw{g}"], np.float32) for g in range(5)]
    nbs = [np.asarray(inputs[f"nb{g}"], np.float32) for g in range(5)]
    fbs = [np.asarray(inputs[f"fb{g}"], np.float32) for g in range(5)]

    cst = np.zeros((128, NCONST), np.float32)

    def put_bf16(name, arr128x128):
        u16 = np.ascontiguousarray(arr128x128.astype(bf16)).view(np.uint16)
        u32 = u16[:, 0::2].astype(np.uint32) | (
            u16[:, 1::2].astype(np.uint32) << 16)
        cst[:, CONST_OFF[name]:CONST_OFF[name] + 64] = u32.view(np.float32)

    c1t = np.zeros((C, NB), np.float32)
    c2t = np.zeros((C, NB), np.float32)
    for j, (q0, w, g, i) in enumerate(BANDS):
        c = CHUNK_OF[j]
        r0 = q0 - CHUNKS[c][0]
        fw, nw, nb, fb = fws[g][i], nws[g][i], nbs[g][i], fbs[g][i]
        fwn = np.zeros((128, C), np.float32)
        fwn[r0:r0 + w, :] = (fw * nw[None, :]).T
        put_bf16(f"fwn{j}", fwn)
        c1t[:, j] = fb + fw @ nb
        c2t[:, j] = fw @ nw
    for c, (q0, rows, bl) in enumerate(CHUNKS):
        ind = np.zeros((128, NB), np.float32)
        for j in bl:
            qb, w, _g, _i = BANDS[j]
            ind[qb - q0:qb - q0 + w, j] = 1.0
        cst[:, CONST_OFF[f"ind{c}"]:CONST_OFF[f"ind{c}"] + NB] = ind
    cst[:, CONST_OFF["c1t"]:CONST_OFF["c1t"] + NB] = c1t
    cst[:, CONST_OFF["c2t"]:CONST_OFF["c2t"] + NB] = c2t
    cst[0:NB, CONST_OFF["ones34"]:CONST_OFF["ones34"] + C] = 1.0
    cst[0:NB, CONST_OFF["eye34"]:CONST_OFF["eye34"] + NB] = np.eye(NB)
    cst[0:NB, CONST_OFF["invc"]] = [1.0 / w for (_q0, w, _g, _i) in BANDS]
    cst[0:NB, CONST_OFF["epsv"]] = EPS
    return {"cst": cst}


def prep_x(x_sample_f32, bf16):
    """(T, Q) f32 -> chunk-padded (T, XQ) bf16."""
    xp = np.zeros((T, XQ), bf16)
    for c, (q0, rows, _bl) in enumerate(CHUNKS):
        xp[:, c * 128:c * 128 + rows] = x_sample_f32[:, q0:q0 + rows].astype(
            bf16)
    return xp


def build_module(phases=4, ntc_cap=99, skip_out_dma=False, out_mode="mix"):
    import concourse.bacc as bacc
    import concourse.tile as tile
    import concourse.mybir as mybir
    from contextlib import ExitStack

    f32 = mybir.dt.float32
    bf16 = mybir.dt.bfloat16
    AF = mybir.ActivationFunctionType
    ALU = mybir.AluOpType
    nc = bacc.Bacc(None)

    x_d = nc.declare_dram_parameter("x", [T, XQ], bf16, isOutput=False)
    cst_d = nc.declare_dram_parameter("cst", [128, NCONST], f32, isOutput=False)
    out_d = nc.declare_dram_parameter("out", [C, T, NB], f32, isOutput=True)

    with tile.TileContext(nc) as tc, ExitStack() as ctx:
        cpool = ctx.enter_context(tc.tile_pool(name="cpool", bufs=1))
        stpool = ctx.enter_context(tc.tile_pool(name="st", bufs=2))
        smpool = ctx.enter_context(tc.tile_pool(name="sm", bufs=4))
        ps_out = ctx.enter_context(tc.tile_pool(name="ps_out", bufs=4, space="PSUM"))
        ps_sm = ctx.enter_context(tc.tile_pool(name="ps_sm", bufs=1, space="PSUM"))

        cst = cpool.tile([128, NCONST], f32, tag="cst", name="cst_t")
        # small late-layout consts (ind..eps) first; fwn block after the
        # x transposes (needed only in the output phase)
        nc.sync.dma_start(cst[:, CST_SPLIT:], cst_d[:, CST_SPLIT:])

        def cview(name, cols, dt=f32, parts=128):
            o = CONST_OFF[name]
            v = cst[0:parts, o:o + (cols if dt is f32 else cols // 2)]
            return v.bitcast(dt) if dt is not f32 else v

        fwp = [cview(f"fwn{j}", 128, bf16) for j in range(NB)]
        ind = [cview(f"ind{c}", NB) for c in range(NCH)]
        c1t = cview("c1t", NB)
        c2t = cview("c2t", NB)
        ones34 = cview("ones34", C, parts=NB)
        eye34 = cview("eye34", NB, parts=NB)
        invc = cview("invc", 1, parts=NB)
        epsap = cview("epsv", 1, parts=NB)

        xT = [cpool.tile([128, T], bf16, tag=f"xT{c}", name=f"xT{c}")
              for c in range(NCH)]
        musig = cpool.tile([NB, 2], f32, tag="musig", name="musig")
        bcast = cpool.tile([C, 2 * NB], f32, tag="bcast", name="bcast")
        btot = cpool.tile([C, NB], f32, tag="btot", name="btot")

        # preload both ACT tables used later (Sqrt for stats, Identity for
        # output copies) so the loads sit off the critical path
        warm = smpool.tile([1, 2], f32, tag="warm", name="warm")
        nc.vector.memset(warm[:], 0.0)
        nc.scalar.activation(warm[:, 0:1], warm[:, 0:1], AF.Sqrt,
                             bias=warm[:, 1:2], scale=1.0)
        nc.scalar.activation(warm[:, 0:1], warm[:, 0:1], AF.Identity,
                             bias=warm[:, 1:2], scale=1.0)

        # ---- L: XBAR DMA-transpose each chunk, then windowed bn_stats ----
        st6 = [smpool.tile([CHUNKS[c][1], 6 * len(WINS)], f32,
                           tag=f"st6_{c}", name=f"st6_{c}", bufs=1)
               for c in range(NCH)]
        for c in range(NCH):
            nc.sync.dma_start_transpose(xT[c][:], x_d[:, c * 128:(c + 1) * 128])
            if phases >= 2:
                rows = CHUNKS[c][1]
                for wi, (ws, wl) in enumerate(WINS):
                    nc.vector.bn_stats(st6[c][:, 6 * wi:6 * (wi + 1)],
                                       xT[c][0:rows, ws:ws + wl])
        nc.sync.dma_start(cst[:, 0:CST_SPLIT], cst_d[:, 0:CST_SPLIT])

        if phases == 1:
            nc.sync.dma_start(out_d[0:CHUNKS[0][1], 0:T // 2, 0],
                              xT[0][0:CHUNKS[0][1], :].bitcast(f32))

        if phases >= 2:
            # ---- S: stats -> mu, rstd -> broadcast + btot ----
            stats_ps = ps_sm.tile([NB, 2], f32, tag="small", name="stats_ps")
            for c in range(NCH):
                rows = CHUNKS[c][1]
                s12 = smpool.tile([rows, 2], f32, tag="s12", name=f"s12_{c}")
                tmp = smpool.tile([rows, 1], f32, tag="tmp", name=f"tmp{c}")
                nc.vector.bn_aggr(s12[:], st6[c][:])
                nc.vector.tensor_mul(tmp[:], s12[:, 0:1], s12[:, 0:1])
                nc.vector.tensor_add(s12[:, 1:2], s12[:, 1:2], tmp[:])
                nc.tensor.matmul(stats_ps[:], ind[c][0:rows, :], s12[:],
                                 start=(c == 0), stop=(c == NCH - 1))

            ex2 = smpool.tile([NB, 1], f32, tag="ex2", name="ex2")
            var_t = smpool.tile([NB, 1], f32, tag="var", name="var_t")
            std_t = smpool.tile([NB, 1], f32, tag="std", name="std_t")
            nc.vector.tensor_scalar_mul(musig[:, 0:1], stats_ps[:, 0:1],
                                        invc[:])
            nc.vector.tensor_scalar_mul(ex2[:], stats_ps[:, 1:2], invc[:])
            nc.vector.tensor_mul(var_t[:], musig[:, 0:1], musig[:, 0:1])
            nc.vector.tensor_sub(var_t[:], ex2[:], var_t[:])
            nc.scalar.activation(std_t[:], var_t[:], AF.Sqrt, bias=epsap[:],
                                 scale=1.0)
            nc.vector.reciprocal(musig[:, 1:2], std_t[:])

            # diag trick: [diag(rstd) | diag(mu*rstd)], then ones34^T @ .
            mrs = smpool.tile([NB, 1], f32, tag="mrs", name="mrs")
            nc.vector.tensor_mul(mrs[:], musig[:, 0:1], musig[:, 1:2])
            dg = smpool.tile([NB, 2 * NB], f32, tag="dg", name="dg")
            nc.vector.tensor_scalar_mul(dg[:, 0:NB], eye34[:], musig[:, 1:2])
            nc.vector.tensor_scalar_mul(dg[:, NB:2 * NB], eye34[:], mrs[:])
            bc_ps = ps_sm.tile([C, 2 * NB], f32, tag="small", name="bc_ps")
            nc.tensor.matmul(bc_ps[:], ones34[:], dg[:], start=True, stop=True)
            nc.vector.tensor_copy(bcast[:], bc_ps[:])
            nc.vector.tensor_mul(btot[:], bcast[:, NB:2 * NB], c2t[:])
            nc.vector.tensor_sub(btot[:], c1t[:], btot[:])

        if phases == 2:
            nc.sync.dma_start(out_d[0:NB, 0, 0:2], musig[:])
            nc.sync.dma_start(out_d[0:C, 1, 0:NB], btot[:])

        if phases >= 3:
            # ---- O: per (t-chunk, band) matmul + fused scale/bias copy ----
            t0 = 0
            for tk, TC in enumerate(TCS[:min(len(TCS), ntc_cap)]):
                stag = stpool.tile([C, max(TCS) * NB], f32, tag="stag",
                                   name=f"stag{tk}")
                sv = stag.rearrange("p (t j) -> p t j", j=NB)
                for j in range(NB):
                    c, base, K = _band_window(j)
                    ops = ps_out.tile([C, TC], f32, tag="outp",
                                      name=f"ops{tk}_{j}")
                    nc.tensor.matmul(ops[:], fwp[j][base:base + K, :],
                                     xT[c][base:base + K, t0:t0 + TC],
                                     start=True, stop=True)
                    use_act = ((j + tk) % 2 < 1 if tk == 0
                               else (j + tk) % 9 < 5)
                    if out_mode == "plain":
                        (nc.scalar.copy if use_act else nc.vector.tensor_copy)(
                            sv[:, 0:TC, j], ops[:])
                    elif out_mode == "act" or (out_mode == "mix" and use_act):
                        nc.scalar.activation(sv[:, 0:TC, j], ops[:],
                                             AF.Identity,
                                             bias=btot[:, j:j + 1],
                                             scale=bcast[:, j:j + 1])
                    else:
                        nc.vector.tensor_scalar(sv[:, 0:TC, j], ops[:],
                                                bcast[:, j:j + 1],
                                                btot[:, j:j + 1],
                                                ALU.mult, ALU.add)
                if not skip_out_dma:
                    nc.sync.dma_start(out_d[:, t0:t0 + TC, :],
                                      sv[:, 0:TC, :])
                t0 += TC

    _finalize(nc)
    return nc


def _finalize(nc):
    import concourse.mybir as mybir
    nc.compile()
    # compile()'s late passes can leave >1-wait instructions, which walrus
    # rejects for some instruction types and hardware mishandles for others.
    nc.generate_event_semaphores()
    nc.codegen_inst_isa_subclasses()
    m2 = mybir.parse_bytes(nc.to_json_bytes())
    for fn in m2.functions:
        for bb in fn.blocks:
            for i in bb.instructions:
                si = i.sync_info
                n = len(si.on_wait) if si and si.on_wait else 0
                assert n <= 1 or type(i).__name__ == "InstEventSemaphore", (
                    f"multi-wait survived: {i.name} {type(i).__name__} {n}")


_CACHE = {}


def _get_module():
    if "nc" not in _CACHE:
        _CACHE["nc"] = build_module()
    return _CACHE["nc"]


def prepare_in_maps(inputs):
    import concourse.mybir as mybir
    bf16 = mybir.dt.np(mybir.dt.bfloat16)
    x = np.ascontiguousarray(
        np.asarray(inputs["x"], dtype=np.float32)).reshape(B, T, Q)
    base = host_constants(inputs)
    return [dict(base, x=prep_x(x[i], bf16)) for i in range(B)]


def kernel(**inputs):
    from concourse.bass_utils import run_bass_kernel_spmd

    nc = _get_module()
    in_maps = prepare_in_maps(inputs)
    res = run_bass_kernel_spmd(nc, in_maps, core_ids=list(range(B)))
    return np.stack([res.results[i]["out"] for i in range(B)], axis=0)


# revision 6
# speedup vs baseline: 1.0226x; 1.0226x over previous
"""BandSplit Trainium2 kernel, v4 (XBAR DMA-transpose load).

Math (per sample b, band j covering flat-channel segment [q0, q0+w)):
  x viewed as (T, 962); GroupNorm over (T, w) per band, then per-band
  1x1 conv:  out_j = fw_j @ xn_j^T + fb_j.

Folding: out[c,t] = rstd_j*(fwn_j @ x)[c,t] + (c1[c,j] - mu_j*rstd_j*c2[c,j])
with fwn = fw*nw (HOST, bf16), c1 = fb + fw@nb, c2 = fw@nw (HOST, f32);
only the scalars mu_j, rstd_j are computed on device and applied in the
PSUM->staging copy as a fused scale+bias.

v4 load path: the host ships x chunk-padded as (T, 10*128) bf16 and each
of the 10 band-aligned chunks is brought into SBUF directly transposed by
the XBAR DMA (InstDmaTransposeAnt, 16x128 tiles; T=2000 = 125*16). This
removes the PE transposes and all PSUM->SBUF copies from the load phase;
bn_stats for a chunk starts as soon as its single DMA lands, so the
stats gate moves ~12us earlier.

Output: per (t-chunk, band) bf16 matmul K=w into PSUM; ACT/DVE fused
copy out = psum*rstd_j + btot[:,j] into band-interleaved staging; one
DMA per t-chunk writes (C, tc, 34) contiguously, graduated chunk sizes
so the write stream starts early and stays dense.
"""
import numpy as np

GROUPS = [(0, 1, 5), (5, 19, 4), (81, 6, 10), (141, 7, 40), (421, 1, 60)]
B, C, T, Q, NB = 8, 128, 2000, 962, 34
EPS = 1e-5
TCS = [100, 200, 300, 400, 400, 400, 200]   # output t-chunks (graduated)
assert sum(TCS) == T
# bn_stats windows (start, len); <=512 (HW limit), count-weighted by aggr
WINS = [(0, 500), (500, 500), (1000, 500), (1500, 500)]

BANDS = []
for _g, (_off, _n, _s) in enumerate(GROUPS):
    for _i in range(_n):
        BANDS.append((2 * _off + _i * 2 * _s, 2 * _s, _g, _i))
assert len(BANDS) == NB and BANDS[-1][0] + BANDS[-1][1] == Q

# band-aligned chunks: consecutive bands packed into <=128 rows
CHUNKS = []   # (q0, rows, [band indices])
_cur, _rows, _q0 = [], 0, 0
for _j, (_qb, _w, _g, _i) in enumerate(BANDS):
    if _rows + _w > 128:
        CHUNKS.append((_q0, _rows, _cur))
        _cur, _rows, _q0 = [], 0, _qb
    _cur.append(_j)
    _rows += _w
CHUNKS.append((_q0, _rows, _cur))
NCH = len(CHUNKS)
CHUNK_OF = {}
for _c, (_q0, _r, _bl) in enumerate(CHUNKS):
    for _j in _bl:
        CHUNK_OF[_j] = _c
XQ = NCH * 128   # chunk-padded x width

# packed const layout (f32 columns); fwn first, late-needed, so the
# small early DMA (ind..eps) can precede it
_off = 0
CONST_OFF = {}
def _creserve(name, cols):
    global _off
    CONST_OFF[name] = _off
    _off += cols
for _j in range(NB):
    _creserve(f"fwn{_j}", 64)             # (128,128) bf16 each
for _c in range(NCH):
    _creserve(f"ind{_c}", NB)             # (128,34) f32
_creserve("c1t", NB)
_creserve("c2t", NB)
_creserve("ones34", C)                    # rows 0..33 used
_creserve("eye34", NB)
_creserve("invc", 1)
_creserve("epsv", 1)
NCONST = _off
CST_SPLIT = CONST_OFF["ind0"]             # [SPLIT:] loads first


def _align(r0, r1):
    """Largest legal quadrant base <= r0 covering [r0, r1)."""
    for base in (64, 32, 0):
        if base > r0:
            continue
        K = r1 - base
        if K <= 32:
            return base, K
        if K <= 64 and base in (0, 64):
            return base, K
        if base == 0:
            return 0, K
    raise AssertionError((r0, r1))


def _band_window(j):
    q0, w, _g, _i = BANDS[j]
    c = CHUNK_OF[j]
    r0 = q0 - CHUNKS[c][0]
    base, K = _align(r0, r0 + w)
    return c, base, K


def host_constants(inputs):
    import concourse.mybir as mybir
    bf16 = mybir.dt.np(mybir.dt.bfloat16)

    fws = [np.asarray(inputs[f"fw{g}"], np.float32) for g in range(5)]
    nws = [np.asarray(inputs[f"nw{g}"], np.float32) for g in range(5)]
    nbs = [np.asarray(inputs[f"nb{g}"], np.float32) for g in range(5)]
    fbs = [np.asarray(inputs[f"fb{g}"], np.float32) for g in range(5)]

    cst = np.zeros((128, NCONST), np.float32)

    def put_bf16(name, arr128x128):
        u16 = np.ascontiguousarray(arr128x128.astype(bf16)).view(np.uint16)
        u32 = u16[:, 0::2].astype(np.uint32) | (
            u16[:, 1::2].astype(np.uint32) << 16)
        cst[:, CONST_OFF[name]:CONST_OFF[name] + 64] = u32.view(np.float32)

    c1t = np.zeros((C, NB), np.float32)
    c2t = np.zeros((C, NB), np.float32)
    for j, (q0, w, g, i) in enumerate(BANDS):
        c = CHUNK_OF[j]
        r0 = q0 - CHUNKS[c][0]
        fw, nw, nb, fb = fws[g][i], nws[g][i], nbs[g][i], fbs[g][i]
        fwn = np.zeros((128, C), np.float32)
        fwn[r0:r0 + w, :] = (fw * nw[None, :]).T
        put_bf16(f"fwn{j}", fwn)
        c1t[:, j] = fb + fw @ nb
        c2t[:, j] = fw @ nw
    for c, (q0, rows, bl) in enumerate(CHUNKS):
        ind = np.zeros((128, NB), np.float32)
        for j in bl:
            qb, w, _g, _i = BANDS[j]
            ind[qb - q0:qb - q0 + w, j] = 1.0
        cst[:, CONST_OFF[f"ind{c}"]:CONST_OFF[f"ind{c}"] + NB] = ind
    cst[:, CONST_OFF["c1t"]:CONST_OFF["c1t"] + NB] = c1t
    cst[:, CONST_OFF["c2t"]:CONST_OFF["c2t"] + NB] = c2t
    cst[0:NB, CONST_OFF["ones34"]:CONST_OFF["ones34"] + C] = 1.0
    cst[0:NB, CONST_OFF["eye34"]:CONST_OFF["eye34"] + NB] = np.eye(NB)
    cst[0:NB, CONST_OFF["invc"]] = [1.0 / w for (_q0, w, _g, _i) in BANDS]
    cst[0:NB, CONST_OFF["epsv"]] = EPS
    return {"cst": cst}


def prep_x(x_sample_f32, bf16):
    """(T, Q) f32 -> chunk-padded (T, XQ) bf16."""
    xp = np.zeros((T, XQ), bf16)
    for c, (q0, rows, _bl) in enumerate(CHUNKS):
        xp[:, c * 128:c * 128 + rows] = x_sample_f32[:, q0:q0 + rows].astype(
            bf16)
    return xp


def build_module(phases=4, ntc_cap=99, skip_out_dma=False, out_mode="mix"):
    import concourse.bacc as bacc
    import concourse.tile as tile
    import concourse.mybir as mybir
    from contextlib import ExitStack

    f32 = mybir.dt.float32
    bf16 = mybir.dt.bfloat16
    AF = mybir.ActivationFunctionType
    ALU = mybir.AluOpType
    nc = bacc.Bacc(None)

    x_d = nc.declare_dram_parameter("x", [T, XQ], bf16, isOutput=False)
    cst_d = nc.declare_dram_parameter("cst", [128, NCONST], f32, isOutput=False)
    out_d = nc.declare_dram_parameter("out", [C, T, NB], f32, isOutput=True)

    with tile.TileContext(nc) as tc, ExitStack() as ctx:
        cpool = ctx.enter_context(tc.tile_pool(name="cpool", bufs=1))
        stpool = ctx.enter_context(tc.tile_pool(name="st", bufs=2))
        smpool = ctx.enter_context(tc.tile_pool(name="sm", bufs=4))
        ps_out = ctx.enter_context(tc.tile_pool(name="ps_out", bufs=4, space="PSUM"))
        ps_sm = ctx.enter_context(tc.tile_pool(name="ps_sm", bufs=1, space="PSUM"))

        cst = cpool.tile([128, NCONST], f32, tag="cst", name="cst_t")
        # small late-layout consts (ind..eps) first; fwn block after the
        # x transposes (needed only in the output phase)
        nc.sync.dma_start(cst[:, CST_SPLIT:], cst_d[:, CST_SPLIT:])

        def cview(name, cols, dt=f32, parts=128):
            o = CONST_OFF[name]
            v = cst[0:parts, o:o + (cols if dt is f32 else cols // 2)]
            return v.bitcast(dt) if dt is not f32 else v

        fwp = [cview(f"fwn{j}", 128, bf16) for j in range(NB)]
        ind = [cview(f"ind{c}", NB) for c in range(NCH)]
        c1t = cview("c1t", NB)
        c2t = cview("c2t", NB)
        ones34 = cview("ones34", C, parts=NB)
        eye34 = cview("eye34", NB, parts=NB)
        invc = cview("invc", 1, parts=NB)
        epsap = cview("epsv", 1, parts=NB)

        xT = [cpool.tile([128, T], bf16, tag=f"xT{c}", name=f"xT{c}")
              for c in range(NCH)]
        musig = cpool.tile([NB, 2], f32, tag="musig", name="musig")
        bcast = cpool.tile([C, 2 * NB], f32, tag="bcast", name="bcast")
        btot = cpool.tile([C, NB], f32, tag="btot", name="btot")

        # preload both ACT tables used later (Sqrt for stats, Identity for
        # output copies) so the loads sit off the critical path
        warm = smpool.tile([1, 2], f32, tag="warm", name="warm")
        nc.vector.memset(warm[:], 0.0)
        nc.scalar.activation(warm[:, 0:1], warm[:, 0:1], AF.Sqrt,
                             bias=warm[:, 1:2], scale=1.0)
        nc.scalar.activation(warm[:, 0:1], warm[:, 0:1], AF.Identity,
                             bias=warm[:, 1:2], scale=1.0)

        # ---- L: XBAR DMA-transpose each chunk, then windowed stats ----
        # ACT computes raw [sum x, sum x^2] for ACTCH chunks (it is idle
        # during the load); DVE bn_stats covers the rest
        ACTCH = (1, 4, 7)
        st6 = {c: smpool.tile([CHUNKS[c][1], 6 * len(WINS)], f32,
                              tag=f"st6_{c}", name=f"st6_{c}", bufs=1)
               for c in range(NCH) if c not in ACTCH}
        acc = {c: smpool.tile([CHUNKS[c][1], 2 * len(WINS)], f32,
                              tag=f"acc_{c}", name=f"acc_{c}", bufs=1)
               for c in ACTCH}
        for c in range(NCH):
            nc.sync.dma_start_transpose(xT[c][:], x_d[:, c * 128:(c + 1) * 128])
            if phases < 2:
                continue
            rows = CHUNKS[c][1]
            if c in ACTCH:
                for wi, (ws, wl) in enumerate(WINS):
                    scr = smpool.tile([rows, max(w[1] for w in WINS)], bf16,
                                      tag="scr", name=f"scr{c}_{wi}", bufs=3)
                    nc.scalar.activation(
                        scr[:, 0:wl], xT[c][0:rows, ws:ws + wl], AF.Copy,
                        accum_out=acc[c][:, 2 * wi:2 * wi + 1])
                    nc.scalar.activation(
                        scr[:, 0:wl], xT[c][0:rows, ws:ws + wl], AF.Square,
                        accum_out=acc[c][:, 2 * wi + 1:2 * wi + 2])
            else:
                for wi, (ws, wl) in enumerate(WINS):
                    nc.vector.bn_stats(st6[c][:, 6 * wi:6 * (wi + 1)],
                                       xT[c][0:rows, ws:ws + wl])
        nc.sync.dma_start(cst[:, 0:CST_SPLIT], cst_d[:, 0:CST_SPLIT])

        if phases == 1:
            nc.sync.dma_start(out_d[0:CHUNKS[0][1], 0:T // 2, 0],
                              xT[0][0:CHUNKS[0][1], :].bitcast(f32))

        if phases >= 2:
            # ---- S: stats -> mu, rstd -> broadcast + btot ----
            stats_ps = ps_sm.tile([NB, 2], f32, tag="small", name="stats_ps")
            for c in range(NCH):
                rows = CHUNKS[c][1]
                s12 = smpool.tile([rows, 2], f32, tag="s12", name=f"s12_{c}")
                if c in ACTCH:
                    # acc = [s1_w0, s2_w0, s1_w1, ...]; strided reduce then /T
                    av = acc[c].rearrange("p (w k) -> p w k", k=2)
                    nc.vector.tensor_reduce(
                        s12[:], av.rearrange("p w k -> p k w"),
                        mybir.AxisListType.X, ALU.add)
                    nc.vector.tensor_scalar_mul(s12[:], s12[:], 1.0 / T)
                else:
                    tmp = smpool.tile([rows, 1], f32, tag="tmp", name=f"tmp{c}")
                    nc.vector.bn_aggr(s12[:], st6[c][:])
                    nc.vector.tensor_mul(tmp[:], s12[:, 0:1], s12[:, 0:1])
                    nc.vector.tensor_add(s12[:, 1:2], s12[:, 1:2], tmp[:])
                nc.tensor.matmul(stats_ps[:], ind[c][0:rows, :], s12[:],
                                 start=(c == 0), stop=(c == NCH - 1))

            ex2 = smpool.tile([NB, 1], f32, tag="ex2", name="ex2")
            var_t = smpool.tile([NB, 1], f32, tag="var", name="var_t")
            std_t = smpool.tile([NB, 1], f32, tag="std", name="std_t")
            nc.vector.tensor_scalar_mul(musig[:, 0:1], stats_ps[:, 0:1],
                                        invc[:])
            nc.vector.tensor_scalar_mul(ex2[:], stats_ps[:, 1:2], invc[:])
            nc.vector.tensor_mul(var_t[:], musig[:, 0:1], musig[:, 0:1])
            nc.vector.tensor_sub(var_t[:], ex2[:], var_t[:])
            nc.scalar.activation(std_t[:], var_t[:], AF.Sqrt, bias=epsap[:],
                                 scale=1.0)
            nc.vector.reciprocal(musig[:, 1:2], std_t[:])

            # diag trick: [diag(rstd) | diag(mu*rstd)], then ones34^T @ .
            mrs = smpool.tile([NB, 1], f32, tag="mrs", name="mrs")
            nc.vector.tensor_mul(mrs[:], musig[:, 0:1], musig[:, 1:2])
            dg = smpool.tile([NB, 2 * NB], f32, tag="dg", name="dg")
            nc.vector.tensor_scalar_mul(dg[:, 0:NB], eye34[:], musig[:, 1:2])
            nc.vector.tensor_scalar_mul(dg[:, NB:2 * NB], eye34[:], mrs[:])
            bc_ps = ps_sm.tile([C, 2 * NB], f32, tag="small", name="bc_ps")
            nc.tensor.matmul(bc_ps[:], ones34[:], dg[:], start=True, stop=True)
            nc.vector.tensor_copy(bcast[:], bc_ps[:])
            nc.vector.tensor_mul(btot[:], bcast[:, NB:2 * NB], c2t[:])
            nc.vector.tensor_sub(btot[:], c1t[:], btot[:])

        if phases == 2:
            nc.sync.dma_start(out_d[0:NB, 0, 0:2], musig[:])
            nc.sync.dma_start(out_d[0:C, 1, 0:NB], btot[:])

        if phases >= 3:
            # ---- O: per (t-chunk, band) matmul + fused scale/bias copy ----
            t0 = 0
            for tk, TC in enumerate(TCS[:min(len(TCS), ntc_cap)]):
                stag = stpool.tile([C, max(TCS) * NB], f32, tag="stag",
                                   name=f"stag{tk}")
                sv = stag.rearrange("p (t j) -> p t j", j=NB)
                for j in range(NB):
                    c, base, K = _band_window(j)
                    ops = ps_out.tile([C, TC], f32, tag="outp",
                                      name=f"ops{tk}_{j}")
                    nc.tensor.matmul(ops[:], fwp[j][base:base + K, :],
                                     xT[c][base:base + K, t0:t0 + TC],
                                     start=True, stop=True)
                    use_act = ((j + tk) % 2 < 1 if tk == 0
                               else (j + tk) % 9 < 5)
                    if out_mode == "plain":
                        (nc.scalar.copy if use_act else nc.vector.tensor_copy)(
                            sv[:, 0:TC, j], ops[:])
                    elif out_mode == "act" or (out_mode == "mix" and use_act):
                        nc.scalar.activation(sv[:, 0:TC, j], ops[:],
                                             AF.Identity,
                                             bias=btot[:, j:j + 1],
                                             scale=bcast[:, j:j + 1])
                    else:
                        nc.vector.tensor_scalar(sv[:, 0:TC, j], ops[:],
                                                bcast[:, j:j + 1],
                                                btot[:, j:j + 1],
                                                ALU.mult, ALU.add)
                if not skip_out_dma:
                    nc.sync.dma_start(out_d[:, t0:t0 + TC, :],
                                      sv[:, 0:TC, :])
                t0 += TC

    _finalize(nc)
    return nc


def _finalize(nc):
    import concourse.mybir as mybir
    nc.compile()
    # compile()'s late passes can leave >1-wait instructions, which walrus
    # rejects for some instruction types and hardware mishandles for others.
    nc.generate_event_semaphores()
    nc.codegen_inst_isa_subclasses()
    m2 = mybir.parse_bytes(nc.to_json_bytes())
    for fn in m2.functions:
        for bb in fn.blocks:
            for i in bb.instructions:
                si = i.sync_info
                n = len(si.on_wait) if si and si.on_wait else 0
                assert n <= 1 or type(i).__name__ == "InstEventSemaphore", (
                    f"multi-wait survived: {i.name} {type(i).__name__} {n}")


_CACHE = {}


def _get_module():
    if "nc" not in _CACHE:
        _CACHE["nc"] = build_module()
    return _CACHE["nc"]


def prepare_in_maps(inputs):
    import concourse.mybir as mybir
    bf16 = mybir.dt.np(mybir.dt.bfloat16)
    x = np.ascontiguousarray(
        np.asarray(inputs["x"], dtype=np.float32)).reshape(B, T, Q)
    base = host_constants(inputs)
    return [dict(base, x=prep_x(x[i], bf16)) for i in range(B)]


def kernel(**inputs):
    from concourse.bass_utils import run_bass_kernel_spmd

    nc = _get_module()
    in_maps = prepare_in_maps(inputs)
    res = run_bass_kernel_spmd(nc, in_maps, core_ids=list(range(B)))
    return np.stack([res.results[i]["out"] for i in range(B)], axis=0)


# revision 7
# speedup vs baseline: 1.1624x; 1.1367x over previous
"""BandSplit Trainium2 kernel, v4 (XBAR DMA-transpose load).

Math (per sample b, band j covering flat-channel segment [q0, q0+w)):
  x viewed as (T, 962); GroupNorm over (T, w) per band, then per-band
  1x1 conv:  out_j = fw_j @ xn_j^T + fb_j.

Folding: out[c,t] = rstd_j*(fwn_j @ x)[c,t] + (c1[c,j] - mu_j*rstd_j*c2[c,j])
with fwn = fw*nw (HOST, bf16), c1 = fb + fw@nb, c2 = fw@nw (HOST, f32);
only the scalars mu_j, rstd_j are computed on device and applied in the
PSUM->staging copy as a fused scale+bias.

v4 load path: the host ships x chunk-padded as (T, 10*128) bf16 and each
of the 10 band-aligned chunks is brought into SBUF directly transposed by
the XBAR DMA (InstDmaTransposeAnt, 16x128 tiles; T=2000 = 125*16). This
removes the PE transposes and all PSUM->SBUF copies from the load phase;
bn_stats for a chunk starts as soon as its single DMA lands, so the
stats gate moves ~12us earlier.

Output: per (t-chunk, band) bf16 matmul K=w into PSUM; ACT/DVE fused
copy out = psum*rstd_j + btot[:,j] into band-interleaved staging; one
DMA per t-chunk writes (C, tc, 34) contiguously, graduated chunk sizes
so the write stream starts early and stays dense.
"""
import numpy as np

GROUPS = [(0, 1, 5), (5, 19, 4), (81, 6, 10), (141, 7, 40), (421, 1, 60)]
B, C, T, Q, NB = 8, 128, 2000, 962, 34
EPS = 1e-5
TCS = [100, 200, 300, 400, 400, 400, 200]   # output t-chunks (graduated)
assert sum(TCS) == T
# bn_stats windows (start, len); <=512 (HW limit), count-weighted by aggr
WINS = [(0, 500), (500, 500), (1000, 500), (1500, 500)]

BANDS = []
for _g, (_off, _n, _s) in enumerate(GROUPS):
    for _i in range(_n):
        BANDS.append((2 * _off + _i * 2 * _s, 2 * _s, _g, _i))
assert len(BANDS) == NB and BANDS[-1][0] + BANDS[-1][1] == Q

# band-aligned chunks: consecutive bands packed into <=128 rows
CHUNKS = []   # (q0, rows, [band indices])
_cur, _rows, _q0 = [], 0, 0
for _j, (_qb, _w, _g, _i) in enumerate(BANDS):
    if _rows + _w > 128:
        CHUNKS.append((_q0, _rows, _cur))
        _cur, _rows, _q0 = [], 0, _qb
    _cur.append(_j)
    _rows += _w
CHUNKS.append((_q0, _rows, _cur))
NCH = len(CHUNKS)
CHUNK_OF = {}
for _c, (_q0, _r, _bl) in enumerate(CHUNKS):
    for _j in _bl:
        CHUNK_OF[_j] = _c
XQ = NCH * 128   # chunk-padded x width

# packed const layout (f32 columns); fwn first, late-needed, so the
# small early DMA (ind..eps) can precede it
_off = 0
CONST_OFF = {}
def _creserve(name, cols):
    global _off
    CONST_OFF[name] = _off
    _off += cols
for _j in range(NB):
    _creserve(f"fwn{_j}", 64)             # (128,128) bf16 each
for _c in range(NCH):
    _creserve(f"ind{_c}", NB)             # (128,34) f32
_creserve("c1t", NB)
_creserve("c2t", NB)
_creserve("ones34", C)                    # rows 0..33 used
_creserve("eye34", NB)
_creserve("invc", 1)
_creserve("epsv", 1)
NCONST = _off
CST_SPLIT = CONST_OFF["ind0"]             # [SPLIT:] loads first


def _align(r0, r1):
    """Largest legal quadrant base <= r0 covering [r0, r1)."""
    for base in (64, 32, 0):
        if base > r0:
            continue
        K = r1 - base
        if K <= 32:
            return base, K
        if K <= 64 and base in (0, 64):
            return base, K
        if base == 0:
            return 0, K
    raise AssertionError((r0, r1))


def _band_window(j):
    q0, w, _g, _i = BANDS[j]
    c = CHUNK_OF[j]
    r0 = q0 - CHUNKS[c][0]
    base, K = _align(r0, r0 + w)
    return c, base, K


def host_constants(inputs):
    import concourse.mybir as mybir
    bf16 = mybir.dt.np(mybir.dt.bfloat16)

    fws = [np.asarray(inputs[f"fw{g}"], np.float32) for g in range(5)]
    nws = [np.asarray(inputs[f"nw{g}"], np.float32) for g in range(5)]
    nbs = [np.asarray(inputs[f"nb{g}"], np.float32) for g in range(5)]
    fbs = [np.asarray(inputs[f"fb{g}"], np.float32) for g in range(5)]

    cst = np.zeros((128, NCONST), np.float32)

    def put_bf16(name, arr128x128):
        u16 = np.ascontiguousarray(arr128x128.astype(bf16)).view(np.uint16)
        u32 = u16[:, 0::2].astype(np.uint32) | (
            u16[:, 1::2].astype(np.uint32) << 16)
        cst[:, CONST_OFF[name]:CONST_OFF[name] + 64] = u32.view(np.float32)

    c1t = np.zeros((C, NB), np.float32)
    c2t = np.zeros((C, NB), np.float32)
    for j, (q0, w, g, i) in enumerate(BANDS):
        c = CHUNK_OF[j]
        r0 = q0 - CHUNKS[c][0]
        fw, nw, nb, fb = fws[g][i], nws[g][i], nbs[g][i], fbs[g][i]
        fwn = np.zeros((128, C), np.float32)
        fwn[r0:r0 + w, :] = (fw * nw[None, :]).T
        put_bf16(f"fwn{j}", fwn)
        c1t[:, j] = fb + fw @ nb
        c2t[:, j] = fw @ nw
    for c, (q0, rows, bl) in enumerate(CHUNKS):
        ind = np.zeros((128, NB), np.float32)
        for j in bl:
            qb, w, _g, _i = BANDS[j]
            ind[qb - q0:qb - q0 + w, j] = 1.0
        cst[:, CONST_OFF[f"ind{c}"]:CONST_OFF[f"ind{c}"] + NB] = ind
    cst[:, CONST_OFF["c1t"]:CONST_OFF["c1t"] + NB] = c1t
    cst[:, CONST_OFF["c2t"]:CONST_OFF["c2t"] + NB] = c2t
    cst[0:NB, CONST_OFF["ones34"]:CONST_OFF["ones34"] + C] = 1.0
    cst[0:NB, CONST_OFF["eye34"]:CONST_OFF["eye34"] + NB] = np.eye(NB)
    cst[0:NB, CONST_OFF["invc"]] = [1.0 / w for (_q0, w, _g, _i) in BANDS]
    cst[0:NB, CONST_OFF["epsv"]] = EPS
    return {"cst": cst}


def prep_x(x_sample_f32, bf16):
    """(T, Q) f32 -> chunk-padded (T, XQ) bf16."""
    xp = np.zeros((T, XQ), bf16)
    for c, (q0, rows, _bl) in enumerate(CHUNKS):
        xp[:, c * 128:c * 128 + rows] = x_sample_f32[:, q0:q0 + rows].astype(
            bf16)
    return xp


def build_module(phases=4, ntc_cap=99, skip_out_dma=False, out_mode="mix"):
    import concourse.bacc as bacc
    import concourse.tile as tile
    import concourse.mybir as mybir
    from contextlib import ExitStack

    f32 = mybir.dt.float32
    bf16 = mybir.dt.bfloat16
    AF = mybir.ActivationFunctionType
    ALU = mybir.AluOpType
    nc = bacc.Bacc(None)

    x_d = nc.declare_dram_parameter("x", [T, XQ], bf16, isOutput=False)
    cst_d = nc.declare_dram_parameter("cst", [128, NCONST], f32, isOutput=False)
    out_d = nc.declare_dram_parameter("out", [C, T, NB], f32, isOutput=True)

    with tile.TileContext(nc) as tc, ExitStack() as ctx:
        cpool = ctx.enter_context(tc.tile_pool(name="cpool", bufs=1))
        stpool = ctx.enter_context(tc.tile_pool(name="st", bufs=2))
        smpool = ctx.enter_context(tc.tile_pool(name="sm", bufs=4))
        ps_out = ctx.enter_context(tc.tile_pool(name="ps_out", bufs=4, space="PSUM"))
        ps_sm = ctx.enter_context(tc.tile_pool(name="ps_sm", bufs=1, space="PSUM"))

        cst = cpool.tile([128, NCONST], f32, tag="cst", name="cst_t")
        # small late-layout consts (ind..eps) first; fwn block after the
        # x transposes (needed only in the output phase)
        nc.sync.dma_start(cst[:, CST_SPLIT:], cst_d[:, CST_SPLIT:])

        def cview(name, cols, dt=f32, parts=128):
            o = CONST_OFF[name]
            v = cst[0:parts, o:o + (cols if dt is f32 else cols // 2)]
            return v.bitcast(dt) if dt is not f32 else v

        fwp = [cview(f"fwn{j}", 128, bf16) for j in range(NB)]
        ind = [cview(f"ind{c}", NB) for c in range(NCH)]
        c1t = cview("c1t", NB)
        c2t = cview("c2t", NB)
        ones34 = cview("ones34", C, parts=NB)
        eye34 = cview("eye34", NB, parts=NB)
        invc = cview("invc", 1, parts=NB)
        epsap = cview("epsv", 1, parts=NB)

        xT = [cpool.tile([128, T], bf16, tag=f"xT{c}", name=f"xT{c}")
              for c in range(NCH)]
        musig = cpool.tile([NB, 2], f32, tag="musig", name="musig")
        bcast = cpool.tile([C, 2 * NB], f32, tag="bcast", name="bcast")
        btot = cpool.tile([C, NB], f32, tag="btot", name="btot")

        # preload both ACT tables used later (Sqrt for stats, Identity for
        # output copies) so the loads sit off the critical path
        warm = smpool.tile([1, 2], f32, tag="warm", name="warm")
        nc.vector.memset(warm[:], 0.0)
        nc.scalar.activation(warm[:, 0:1], warm[:, 0:1], AF.Sqrt,
                             bias=warm[:, 1:2], scale=1.0)
        nc.scalar.activation(warm[:, 0:1], warm[:, 0:1], AF.Identity,
                             bias=warm[:, 1:2], scale=1.0)

        # ---- L: XBAR DMA-transpose each chunk, then windowed stats ----
        # ACT computes raw [sum x, sum x^2] for ACTCH chunks (it is idle
        # during the load); DVE bn_stats covers the rest
        ACTCH = (1, 5)
        st6 = {c: smpool.tile([CHUNKS[c][1], 6 * len(WINS)], f32,
                              tag=f"st6_{c}", name=f"st6_{c}", bufs=1)
               for c in range(NCH) if c not in ACTCH}
        acc = {c: smpool.tile([CHUNKS[c][1], 2 * len(WINS)], f32,
                              tag=f"acc_{c}", name=f"acc_{c}", bufs=1)
               for c in ACTCH}
        for c in range(NCH):
            nc.sync.dma_start_transpose(xT[c][:], x_d[:, c * 128:(c + 1) * 128])
            if phases < 2:
                continue
            rows = CHUNKS[c][1]
            if c in ACTCH:
                for wi, (ws, wl) in enumerate(WINS):
                    scr = smpool.tile([rows, max(w[1] for w in WINS)], bf16,
                                      tag="scr", name=f"scr{c}_{wi}", bufs=3)
                    nc.scalar.activation(
                        scr[:, 0:wl], xT[c][0:rows, ws:ws + wl], AF.Copy,
                        accum_out=acc[c][:, 2 * wi:2 * wi + 1])
                    nc.scalar.activation(
                        scr[:, 0:wl], xT[c][0:rows, ws:ws + wl], AF.Square,
                        accum_out=acc[c][:, 2 * wi + 1:2 * wi + 2])
            else:
                for wi, (ws, wl) in enumerate(WINS):
                    nc.vector.bn_stats(st6[c][:, 6 * wi:6 * (wi + 1)],
                                       xT[c][0:rows, ws:ws + wl])
        nc.sync.dma_start(cst[:, 0:CST_SPLIT], cst_d[:, 0:CST_SPLIT])

        if phases == 1:
            nc.sync.dma_start(out_d[0:CHUNKS[0][1], 0:T // 2, 0],
                              xT[0][0:CHUNKS[0][1], :].bitcast(f32))

        if phases >= 2:
            # ---- S: stats -> mu, rstd -> broadcast + btot ----
            stats_ps = ps_sm.tile([NB, 2], f32, tag="small", name="stats_ps")
            for c in range(NCH):
                rows = CHUNKS[c][1]
                s12 = smpool.tile([rows, 2], f32, tag="s12", name=f"s12_{c}")
                if c in ACTCH:
                    # acc = [s1_w0, s2_w0, s1_w1, ...]; strided reduce then /T
                    av = acc[c].rearrange("p (w k) -> p w k", k=2)
                    nc.vector.tensor_reduce(
                        s12[:], av.rearrange("p w k -> p k w"),
                        mybir.AxisListType.X, ALU.add)
                    nc.vector.tensor_scalar_mul(s12[:], s12[:], 1.0 / T)
                else:
                    tmp = smpool.tile([rows, 1], f32, tag="tmp", name=f"tmp{c}")
                    nc.vector.bn_aggr(s12[:], st6[c][:])
                    nc.vector.tensor_mul(tmp[:], s12[:, 0:1], s12[:, 0:1])
                    nc.vector.tensor_add(s12[:, 1:2], s12[:, 1:2], tmp[:])
                nc.tensor.matmul(stats_ps[:], ind[c][0:rows, :], s12[:],
                                 start=(c == 0), stop=(c == NCH - 1))

            ex2 = smpool.tile([NB, 1], f32, tag="ex2", name="ex2")
            var_t = smpool.tile([NB, 1], f32, tag="var", name="var_t")
            std_t = smpool.tile([NB, 1], f32, tag="std", name="std_t")
            nc.vector.tensor_scalar_mul(musig[:, 0:1], stats_ps[:, 0:1],
                                        invc[:])
            nc.vector.tensor_scalar_mul(ex2[:], stats_ps[:, 1:2], invc[:])
            nc.vector.tensor_mul(var_t[:], musig[:, 0:1], musig[:, 0:1])
            nc.vector.tensor_sub(var_t[:], ex2[:], var_t[:])
            nc.scalar.activation(std_t[:], var_t[:], AF.Sqrt, bias=epsap[:],
                                 scale=1.0)
            nc.vector.reciprocal(musig[:, 1:2], std_t[:])

            # diag trick: [diag(rstd) | diag(mu*rstd)], then ones34^T @ .
            mrs = smpool.tile([NB, 1], f32, tag="mrs", name="mrs")
            nc.vector.tensor_mul(mrs[:], musig[:, 0:1], musig[:, 1:2])
            dg = smpool.tile([NB, 2 * NB], f32, tag="dg", name="dg")
            nc.vector.tensor_scalar_mul(dg[:, 0:NB], eye34[:], musig[:, 1:2])
            nc.vector.tensor_scalar_mul(dg[:, NB:2 * NB], eye34[:], mrs[:])
            bc_ps = ps_sm.tile([C, 2 * NB], f32, tag="small", name="bc_ps")
            nc.tensor.matmul(bc_ps[:], ones34[:], dg[:], start=True, stop=True)
            nc.vector.tensor_copy(bcast[:], bc_ps[:])
            nc.vector.tensor_mul(btot[:], bcast[:, NB:2 * NB], c2t[:])
            nc.vector.tensor_sub(btot[:], c1t[:], btot[:])

        if phases == 2:
            nc.sync.dma_start(out_d[0:NB, 0, 0:2], musig[:])
            nc.sync.dma_start(out_d[0:C, 1, 0:NB], btot[:])

        if phases >= 3:
            # ---- O: per (t-chunk, band) matmul + fused scale/bias copy ----
            t0 = 0
            for tk, TC in enumerate(TCS[:min(len(TCS), ntc_cap)]):
                stag = stpool.tile([C, max(TCS) * NB], f32, tag="stag",
                                   name=f"stag{tk}")
                sv = stag.rearrange("p (t j) -> p t j", j=NB)
                for j in range(NB):
                    c, base, K = _band_window(j)
                    ops = ps_out.tile([C, TC], f32, tag="outp",
                                      name=f"ops{tk}_{j}")
                    nc.tensor.matmul(ops[:], fwp[j][base:base + K, :],
                                     xT[c][base:base + K, t0:t0 + TC],
                                     start=True, stop=True)
                    use_act = ((j + tk) % 2 < 1 if tk == 0
                               else (j + tk) % 9 < 5)
                    if out_mode == "plain":
                        (nc.scalar.copy if use_act else nc.vector.tensor_copy)(
                            sv[:, 0:TC, j], ops[:])
                    elif out_mode == "act" or (out_mode == "mix" and use_act):
                        nc.scalar.activation(sv[:, 0:TC, j], ops[:],
                                             AF.Identity,
                                             bias=btot[:, j:j + 1],
                                             scale=bcast[:, j:j + 1])
                    else:
                        nc.vector.tensor_scalar(sv[:, 0:TC, j], ops[:],
                                                bcast[:, j:j + 1],
                                                btot[:, j:j + 1],
                                                ALU.mult, ALU.add)
                if not skip_out_dma:
                    nc.sync.dma_start(out_d[:, t0:t0 + TC, :],
                                      sv[:, 0:TC, :])
                t0 += TC

    _finalize(nc)
    return nc


def _finalize(nc):
    import concourse.mybir as mybir
    nc.compile()
    # compile()'s late passes can leave >1-wait instructions, which walrus
    # rejects for some instruction types and hardware mishandles for others.
    nc.generate_event_semaphores()
    nc.codegen_inst_isa_subclasses()
    m2 = mybir.parse_bytes(nc.to_json_bytes())
    for fn in m2.functions:
        for bb in fn.blocks:
            for i in bb.instructions:
                si = i.sync_info
                n = len(si.on_wait) if si and si.on_wait else 0
                assert n <= 1 or type(i).__name__ == "InstEventSemaphore", (
                    f"multi-wait survived: {i.name} {type(i).__name__} {n}")


_CACHE = {}


def _get_module():
    if "nc" not in _CACHE:
        _CACHE["nc"] = build_module()
    return _CACHE["nc"]


def prepare_in_maps(inputs):
    import concourse.mybir as mybir
    bf16 = mybir.dt.np(mybir.dt.bfloat16)
    x = np.ascontiguousarray(
        np.asarray(inputs["x"], dtype=np.float32)).reshape(B, T, Q)
    base = host_constants(inputs)
    return [dict(base, x=prep_x(x[i], bf16)) for i in range(B)]


def kernel(**inputs):
    from concourse.bass_utils import run_bass_kernel_spmd

    nc = _get_module()
    in_maps = prepare_in_maps(inputs)
    res = run_bass_kernel_spmd(nc, in_maps, core_ids=list(range(B)))
    return np.stack([res.results[i]["out"] for i in range(B)], axis=0)
